# revision 9
# baseline (speedup 1.0000x reference)
"""Causal multi-head attention block (B=4, S=1024, E=1024, H=16, D=64) on 8 TRN2 cores.

Sharding: data-parallel over batch (4) x tensor-parallel over heads (2 groups of 8).
Core i handles batch i//2, head-group i%2. Each core computes its partial output
projection (row-parallel W_proj); the host sums the two TP partials per batch and
applies the (exact) bias corrections.

Device-side math per core (bf16 compute, f32 accumulate):
  qT = (Wq_g)^T x^T + bq_g          [512, 1024]  (head-major rows h*64+d)
  kT = (Wk_g)^T x^T + bk_g          [512, 1024]
  v  = x Wv_g                       [1024, 512]  (+ ones column per head -> denominator)
  For each head h: PT[sk, sq] = exp((kT_h^T qT_h)/8) * causal_mask (lower blocks only)
  o2T_h[d, sq] = sum_sk v_h[sk, d] * PT[sk, sq];  denom[sq] = ones-row
  o2T_h /= denom  (softmax normalize; no max subtraction -- logits are O(1))
  out_partial = o2T^T Wp_g          [1024, 1024]
Host: out[b] = out_partial[2b] + out_partial[2b+1] + (bv_0 Wp_0 + bv_1 Wp_1 + b_proj)
(the v-bias term is exact because softmax rows sum to 1).
"""

import numpy as np
import ml_dtypes

import concourse.bass as bass
import concourse.tile as tile
from concourse import bacc, mybir
from concourse.bass_utils import run_bass_kernel_spmd
from concourse.masks import make_upper_triangular

BF16 = mybir.dt.bfloat16
F32 = mybir.dt.float32

B, S, E = 4, 1024, 1024
H_TOT, D = 16, 64
NCORES = 8
HL = 8            # heads per core
JL = HL * D       # 512 local qkv dim
P = 128
ET = E // P       # 8 k-tiles over embed dim
JT = JL // P      # 4 partition-tiles over local qkv dim

_NC_CACHE = None


def build_nc():
    nc = bacc.Bacc()

    xT = nc.declare_dram_parameter("xT", [E, S], BF16, isOutput=False)
    wq = nc.declare_dram_parameter("wq", [E, JL], BF16, isOutput=False)
    wk = nc.declare_dram_parameter("wk", [E, JL], BF16, isOutput=False)
    wv = nc.declare_dram_parameter("wv", [E, JL], BF16, isOutput=False)
    wp = nc.declare_dram_parameter("wp", [JL, E], BF16, isOutput=False)
    bq = nc.declare_dram_parameter("bq", [P, JT], F32, isOutput=False)
    bk = nc.declare_dram_parameter("bk", [P, JT], F32, isOutput=False)
    out = nc.declare_dram_parameter("out", [S, E], F32, isOutput=True)

    with tile.TileContext(nc) as tc:
        with (
            tc.tile_pool(name="singles", bufs=1) as singles,
            tc.tile_pool(name="pt", bufs=2) as pt_pool,
            tc.tile_pool(name="rec", bufs=2) as rec_pool,
            tc.tile_pool(name="bc", bufs=2) as bc_pool,
            tc.tile_pool(name="outst", bufs=3) as out_pool,
            tc.tile_pool(name="ps_mm", bufs=2, space="PSUM") as ps_mm,
            tc.tile_pool(name="ps_l", bufs=2, space="PSUM") as ps_l,
            tc.tile_pool(name="ps_o", bufs=2, space="PSUM") as ps_o,
        ):
            # ---- static inputs -> SBUF ----
            xT_sb = singles.tile([P, ET, S], BF16)
            nc.sync.dma_start(out=xT_sb[:], in_=xT[:, :].rearrange("(o p) s -> p o s", p=P))
            wq_sb = singles.tile([P, ET, JL], BF16)
            nc.sync.dma_start(out=wq_sb[:], in_=wq[:, :].rearrange("(o p) j -> p o j", p=P))
            wk_sb = singles.tile([P, ET, JL], BF16)
            nc.sync.dma_start(out=wk_sb[:], in_=wk[:, :].rearrange("(o p) j -> p o j", p=P))
            wv_sb = singles.tile([P, ET, JL], BF16)
            nc.sync.dma_start(out=wv_sb[:], in_=wv[:, :].rearrange("(o p) j -> p o j", p=P))
            wp_sb = singles.tile([P, JT, E], BF16)
            nc.sync.dma_start(out=wp_sb[:], in_=wp[:, :].rearrange("(o p) e -> p o e", p=P))
            bq_sb = singles.tile([P, JT], F32)
            nc.sync.dma_start(out=bq_sb[:], in_=bq[:, :])
            bk_sb = singles.tile([P, JT], F32)
            nc.sync.dma_start(out=bk_sb[:], in_=bk[:, :])

            # causal keep-mask for diagonal PT blocks: 1 where sq >= sk else 0
            mask_sb = singles.tile([P, P], BF16)
            make_upper_triangular(nc, mask_sb[:], val=1.0, diag=True)

            # ---- QKV projections ----
            qT_sb = singles.tile([P, JT, S], BF16)   # row j = h*64+d, head-major
            kT_sb = singles.tile([P, JT, S], BF16)
            o2T_sb = singles.tile([P, JT, S], BF16)  # normalized attn out, same row layout
            vaug_sb = singles.tile([P, ET, HL, D + 1], BF16)  # [sk_p, sk_tile, head, d|ones]
            nc.vector.memset(vaug_sb[:, :, :, D:D + 1], 1.0)

            for jt in range(JT):
                for w_sb, b_sb, dst in ((wq_sb, bq_sb, qT_sb), (wk_sb, bk_sb, kT_sb)):
                    for nb in range(2):
                        ps = ps_mm.tile([P, 512], F32, tag="mm")
                        for kt in range(ET):
                            nc.tensor.matmul(
                                ps[:],
                                lhsT=w_sb[:, kt, jt * P:(jt + 1) * P],
                                rhs=xT_sb[:, kt, nb * 512:(nb + 1) * 512],
                                start=(kt == 0), stop=(kt == ET - 1),
                            )
                        nc.vector.tensor_scalar_add(
                            dst[:, jt, nb * 512:(nb + 1) * 512], ps[:], b_sb[:, jt:jt + 1]
                        )
            for st in range(ET):
                ps = ps_mm.tile([P, 512], F32, tag="mm")
                for kt in range(ET):
                    nc.tensor.matmul(
                        ps[:],
                        lhsT=xT_sb[:, kt, st * P:(st + 1) * P],
                        rhs=wv_sb[:, kt, :],
                        start=(kt == 0), stop=(kt == ET - 1),
                    )
                nc.vector.tensor_copy(
                    out=vaug_sb[:, st, :, 0:D],
                    in_=ps[:].rearrange("p (h d) -> p h d", h=HL),
                )

            # ---- attention, software-pipelined one head ahead so the PE
            # never idles waiting on ScalarE's exp (keeps HAM at 8/8) ----
            def emit_qk(h):
                jt0, po = h // 2, (h % 2) * 64
                qh = qT_sb[po:po + 64, jt0, :]   # [64, S]
                kh = kT_sb[po:po + 64, jt0, :]
                pT = pt_pool.tile([P, ET, S], BF16)  # [sk_p, sk_tile, sq]
                for t in range(ET):
                    lo = t * P
                    # logits tile spans both 512-wide PSUM banks; matmuls write
                    # bank-aligned chunks, one big exp reads across banks
                    psl = ps_l.tile([P, 1024], F32, tag="psl")
                    for cb in range(2):
                        c0, c1 = cb * 512, (cb + 1) * 512
                        s0 = max(lo, c0)
                        if s0 >= c1:
                            continue
                        nc.tensor.matmul(
                            psl[:, s0:c1],
                            lhsT=kh[:, lo:lo + P],
                            rhs=qh[:, s0:c1],
                            start=True, stop=True,
                        )
                    nc.scalar.activation(
                        out=pT[:, t, lo:S], in_=psl[:, lo:S],
                        func=mybir.ActivationFunctionType.Exp, scale=0.125,
                    )
                    # mask the diagonal block (upper-triangular keep)
                    nc.vector.tensor_mul(
                        out=pT[:, t, lo:lo + P], in0=pT[:, t, lo:lo + P], in1=mask_sb[:]
                    )
                return pT

            def emit_pv(h, pT):
                jt0, po = h // 2, (h % 2) * 64
                for sqb in range(2):
                    c0, c1 = sqb * 512, (sqb + 1) * 512
                    pso = ps_o.tile([P, 512], F32, tag="pso")
                    ts = [t for t in range(ET) if t * P < c1]
                    for i, t in enumerate(ts):
                        s0 = max(t * P, c0)
                        off = s0 - c0
                        nc.tensor.matmul(
                            pso[:D + 1, off:512],
                            lhsT=vaug_sb[:, t, h, :],
                            rhs=pT[:, t, s0:c1],
                            start=(i == 0), stop=(i == len(ts) - 1),
                            skip_group_check=True,
                        )
                    # normalize: o2T_h[:, c0:c1] = pso[:64] / pso[64]
                    rec = rec_pool.tile([P, 512], F32)
                    nc.vector.reciprocal(out=rec[:1, :], in_=pso[D:D + 1, :])
                    bcst = bc_pool.tile([P, 512], F32)
                    nc.gpsimd.partition_broadcast(bcst[:64, :], rec[:1, :])
                    nc.vector.tensor_mul(
                        out=o2T_sb[po:po + 64, jt0, c0:c1],
                        in0=pso[:64, :], in1=bcst[:64, :],
                    )

            pT_prev = emit_qk(0)
            for h in range(1, HL):
                pT_cur = emit_qk(h)
                emit_pv(h - 1, pT_prev)
                pT_prev = pT_cur
            emit_pv(HL - 1, pT_prev)

            # ---- output projection (partial over local heads) ----
            for st in range(S // P):
                for eb in range(2):
                    psf = ps_mm.tile([P, 512], F32, tag="mm")
                    for kt in range(JT):
                        nc.tensor.matmul(
                            psf[:],
                            lhsT=o2T_sb[:, kt, st * P:(st + 1) * P],
                            rhs=wp_sb[:, kt, eb * 512:(eb + 1) * 512],
                            start=(kt == 0), stop=(kt == JT - 1),
                        )
                    ob = out_pool.tile([P, 512], F32)
                    nc.vector.tensor_copy(out=ob[:], in_=psf[:])
                    nc.sync.dma_start(
                        out=out[st * P:(st + 1) * P, eb * 512:(eb + 1) * 512], in_=ob[:]
                    )

    nc.compile()
    return nc


def make_in_maps(x, W_attn, b_attn, W_proj, b_proj):
    bf16 = ml_dtypes.bfloat16
    in_maps = []
    for i in range(NCORES):
        b, g = i // 2, i % 2
        j0 = g * JL
        in_maps.append({
            "xT": np.ascontiguousarray(x[b].T).astype(bf16),
            "wq": W_attn[:, j0:j0 + JL].astype(bf16),
            "wk": W_attn[:, E + j0:E + j0 + JL].astype(bf16),
            "wv": W_attn[:, 2 * E + j0:2 * E + j0 + JL].astype(bf16),
            "wp": W_proj[j0:j0 + JL, :].astype(bf16),
            "bq": np.ascontiguousarray(
                b_attn[j0:j0 + JL].astype(np.float32).reshape(JT, P).T),
            "bk": np.ascontiguousarray(
                b_attn[E + j0:E + j0 + JL].astype(np.float32).reshape(JT, P).T),
        })
    return in_maps


def kernel(x, W_attn, b_attn, W_proj, b_proj):
    global _NC_CACHE
    if _NC_CACHE is None:
        _NC_CACHE = build_nc()
    nc = _NC_CACHE

    in_maps = make_in_maps(x, W_attn, b_attn, W_proj, b_proj)
    res = run_bass_kernel_spmd(nc, in_maps, core_ids=list(range(NCORES)))

    # host unshard: sum the two head-group partials + exact bias corrections
    bias_row = b_proj.astype(np.float32).copy()
    for g in range(2):
        j0 = g * JL
        bv = b_attn[2 * E + j0:2 * E + j0 + JL].astype(np.float32)
        bias_row += bv @ W_proj[j0:j0 + JL, :].astype(np.float32)

    full = np.empty((B, S, E), np.float32)
    for b in range(B):
        full[b] = (res.results[2 * b]["out"] + res.results[2 * b + 1]["out"]
                   + bias_row[None, :])
    return full


# revision 17
# speedup vs baseline: 1.3130x; 1.3130x over previous
"""Causal multi-head attention block (B=4, S=1024, E=1024, H=16, D=64) on 8 TRN2 cores.

Sharding: data-parallel over batch (4) x tensor-parallel over heads (2 groups of 8).
Core i handles batch i//2, head-group i%2. Each core computes its partial output
projection (row-parallel W_proj); the host sums the two TP partials per batch and
applies the (exact) bias corrections.

Device-side math per core (bf16 compute, f32 accumulate):
  qT = (Wq_g)^T x^T + bq_g          [512, 1024]  (head-major rows h*64+d)
  kT = (Wk_g)^T x^T + bk_g          [512, 1024]
  v  = x Wv_g                       [1024, 512]  (+ ones column per head -> denominator)
  For each head h: PT[sk, sq] = exp((kT_h^T qT_h)/8) * causal_mask (lower blocks only)
  o2T_h[d, sq] = sum_sk v_h[sk, d] * PT[sk, sq];  denom[sq] = ones-row
  o2T_h /= denom  (softmax normalize; no max subtraction -- logits are O(1))
  out_partial = o2T^T Wp_g          [1024, 1024]
Host: out[b] = out_partial[2b] + out_partial[2b+1] + (bv_0 Wp_0 + bv_1 Wp_1 + b_proj)
(the v-bias term is exact because softmax rows sum to 1).
"""

import numpy as np
import ml_dtypes

import concourse.bass as bass
import concourse.tile as tile
from concourse import bacc, mybir
from concourse.bass_utils import run_bass_kernel_spmd
from concourse.masks import make_upper_triangular

BF16 = mybir.dt.bfloat16
F32 = mybir.dt.float32

B, S, E = 4, 1024, 1024
H_TOT, D = 16, 64
NCORES = 8
HL = 8            # heads per core
JL = HL * D       # 512 local qkv dim
P = 128
ET = E // P       # 8 k-tiles over embed dim
JT = JL // P      # 4 partition-tiles over local qkv dim

_NC_CACHE = None


def build_nc():
    nc = bacc.Bacc()

    xT = nc.declare_dram_parameter("xT", [E, S], BF16, isOutput=False)
    wq = nc.declare_dram_parameter("wq", [E, JL], BF16, isOutput=False)
    wk = nc.declare_dram_parameter("wk", [E, JL], BF16, isOutput=False)
    wv = nc.declare_dram_parameter("wv", [E, JL], BF16, isOutput=False)
    wp = nc.declare_dram_parameter("wp", [JL, E], BF16, isOutput=False)
    bq = nc.declare_dram_parameter("bq", [P, JT], F32, isOutput=False)
    bk = nc.declare_dram_parameter("bk", [P, JT], F32, isOutput=False)
    out = nc.declare_dram_parameter("out", [S, E], F32, isOutput=True)

    with tile.TileContext(nc) as tc:
        with (
            tc.tile_pool(name="singles", bufs=1) as singles,
            tc.tile_pool(name="pt", bufs=2) as pt_pool,
            tc.tile_pool(name="rec", bufs=2) as rec_pool,
            tc.tile_pool(name="bc", bufs=2) as bc_pool,
            tc.tile_pool(name="outst", bufs=3) as out_pool,
            tc.tile_pool(name="ps_mm", bufs=2, space="PSUM") as ps_mm,
            tc.tile_pool(name="ps_l", bufs=2, space="PSUM") as ps_l,
            tc.tile_pool(name="ps_o", bufs=2, space="PSUM") as ps_o,
        ):
            # ---- static inputs -> SBUF (per-ktile DMAs so the first QKV
            # chain can start as soon as its first tiles land) ----
            xT_sb = singles.tile([P, ET, S], BF16)
            wq_sb = singles.tile([P, ET, JL], BF16)
            wk_sb = singles.tile([P, ET, JL], BF16)
            wv_sb = singles.tile([P, ET, JL], BF16)
            xT_r = xT[:, :].rearrange("(o p) s -> p o s", p=P)
            wq_r = wq[:, :].rearrange("(o p) j -> p o j", p=P)
            wk_r = wk[:, :].rearrange("(o p) j -> p o j", p=P)
            wv_r = wv[:, :].rearrange("(o p) j -> p o j", p=P)
            bq_sb = singles.tile([P, JT], F32)
            nc.sync.dma_start(out=bq_sb[:], in_=bq[:, :])
            bk_sb = singles.tile([P, JT], F32)
            nc.sync.dma_start(out=bk_sb[:], in_=bk[:, :])
            for kt in range(ET):
                nc.sync.dma_start(out=wq_sb[:, kt], in_=wq_r[:, kt])
                nc.sync.dma_start(out=xT_sb[:, kt], in_=xT_r[:, kt])
            for kt in range(ET):
                nc.sync.dma_start(out=wk_sb[:, kt], in_=wk_r[:, kt])
            for kt in range(ET):
                nc.sync.dma_start(out=wv_sb[:, kt], in_=wv_r[:, kt])
            wp_sb = singles.tile([P, JT, E], BF16)
            nc.sync.dma_start(out=wp_sb[:], in_=wp[:, :].rearrange("(o p) e -> p o e", p=P))

            # causal keep-mask for diagonal PT blocks: 1 where sq >= sk else 0
            mask_sb = singles.tile([P, P], BF16)
            make_upper_triangular(nc, mask_sb[:], val=1.0, diag=True)

            # ---- QKV projections ----
            qT_sb = singles.tile([P, JT, S], BF16)   # row j = h*64+d, head-major
            kT_sb = singles.tile([P, JT, S], BF16)
            o2T_sb = singles.tile([P, JT, S], BF16)  # normalized attn out, same row layout
            vaug_sb = singles.tile([P, ET, HL, D + 1], BF16)  # [sk_p, sk_tile, head, d|ones]
            nc.vector.memset(vaug_sb[:, :, :, D:D + 1], 1.0)

            for jt in range(JT):
                for w_sb, b_sb, dst in ((wq_sb, bq_sb, qT_sb), (wk_sb, bk_sb, kT_sb)):
                    for nb in range(2):
                        ps = ps_mm.tile([P, 512], F32, tag="mm")
                        for kt in range(ET):
                            nc.tensor.matmul(
                                ps[:],
                                lhsT=w_sb[:, kt, jt * P:(jt + 1) * P],
                                rhs=xT_sb[:, kt, nb * 512:(nb + 1) * 512],
                                start=(kt == 0), stop=(kt == ET - 1),
                            )
                        # ScalarE is idle during the QKV phase; do the
                        # psum->sbuf bias-add+cast there, not on DVE
                        nc.scalar.activation(
                            out=dst[:, jt, nb * 512:(nb + 1) * 512], in_=ps[:],
                            func=mybir.ActivationFunctionType.Identity,
                            bias=b_sb[:, jt:jt + 1],
                        )
            for st in range(ET):
                ps = ps_mm.tile([P, 512], F32, tag="mm")
                for kt in range(ET):
                    nc.tensor.matmul(
                        ps[:],
                        lhsT=xT_sb[:, kt, st * P:(st + 1) * P],
                        rhs=wv_sb[:, kt, :],
                        start=(kt == 0), stop=(kt == ET - 1),
                    )
                nc.scalar.copy(
                    out=vaug_sb[:, st, :, 0:D],
                    in_=ps[:].rearrange("p (h d) -> p h d", h=HL),
                )

            # ---- attention, software-pipelined one head ahead so the PE
            # never idles waiting on ScalarE's exp (keeps HAM at 8/8) ----
            def emit_qk(h):
                jt0, po = h // 2, (h % 2) * 64
                qh = qT_sb[po:po + 64, jt0, :]   # [64, S]
                kh = kT_sb[po:po + 64, jt0, :]
                pT = pt_pool.tile([P, ET, S], BF16)  # [sk_p, sk_tile, sq]
                for t in range(ET):
                    lo = t * P
                    # logits tile spans both 512-wide PSUM banks; matmuls write
                    # bank-aligned chunks, one big exp reads across banks
                    psl = ps_l.tile([P, 1024], F32, tag="psl")
                    for cb in range(2):
                        c0, c1 = cb * 512, (cb + 1) * 512
                        s0 = max(lo, c0)
                        if s0 >= c1:
                            continue
                        nc.tensor.matmul(
                            psl[:, s0:c1],
                            lhsT=kh[:, lo:lo + P],
                            rhs=qh[:, s0:c1],
                            start=True, stop=True,
                        )
                    nc.scalar.activation(
                        out=pT[:, t, lo:S], in_=psl[:, lo:S],
                        func=mybir.ActivationFunctionType.Exp, scale=0.125,
                    )
                # mask all 8 diagonal blocks in one strided multiply:
                # block t lives at free offset t*(S+P) in the flattened tile
                diag = bass.AP(tensor=pT.tensor, offset=pT.offset,
                               ap=[list(pT.ap[0]), [S + P, ET], [1, P]])
                nc.vector.tensor_mul(
                    out=diag, in0=diag,
                    in1=mask_sb[:, None, :].to_broadcast([P, ET, P]),
                )
                return pT

            def emit_pv(h, pT):
                jt0, po = h // 2, (h % 2) * 64
                for sqb in range(2):
                    c0, c1 = sqb * 512, (sqb + 1) * 512
                    pso = ps_o.tile([P, 512], F32, tag="pso")
                    ts = [t for t in range(ET) if t * P < c1]
                    for i, t in enumerate(ts):
                        s0 = max(t * P, c0)
                        off = s0 - c0
                        nc.tensor.matmul(
                            pso[:D + 1, off:512],
                            lhsT=vaug_sb[:, t, h, :],
                            rhs=pT[:, t, s0:c1],
                            start=(i == 0), stop=(i == len(ts) - 1),
                            skip_group_check=True,
                        )
                    # normalize: o2T_h[:, c0:c1] = pso[:64] / pso[64]
                    rec = rec_pool.tile([P, 512], F32)
                    # custom-DVE op wants SBUF input at partition 0: stage the
                    # denominator row first, then approximate in place
                    nc.vector.tensor_copy(out=rec[:1, :], in_=pso[D:D + 1, :])
                    nc.vector.reciprocal_approx_fast(out=rec[:1, :], in_=rec[:1, :])
                    bcst = bc_pool.tile([P, 512], F32)
                    nc.gpsimd.partition_broadcast(bcst[:64, :], rec[:1, :])
                    nc.vector.tensor_mul(
                        out=o2T_sb[po:po + 64, jt0, c0:c1],
                        in0=pso[:64, :], in1=bcst[:64, :],
                    )

            pT_prev = emit_qk(0)
            for h in range(1, HL):
                pT_cur = emit_qk(h)
                emit_pv(h - 1, pT_prev)
                pT_prev = pT_cur
            emit_pv(HL - 1, pT_prev)

            # ---- output projection (partial over local heads) ----
            for st in range(S // P):
                for eb in range(2):
                    psf = ps_mm.tile([P, 512], F32, tag="mm")
                    for kt in range(JT):
                        nc.tensor.matmul(
                            psf[:],
                            lhsT=o2T_sb[:, kt, st * P:(st + 1) * P],
                            rhs=wp_sb[:, kt, eb * 512:(eb + 1) * 512],
                            start=(kt == 0), stop=(kt == JT - 1),
                        )
                    ob = out_pool.tile([P, 512], F32)
                    nc.scalar.copy(out=ob[:], in_=psf[:])
                    nc.sync.dma_start(
                        out=out[st * P:(st + 1) * P, eb * 512:(eb + 1) * 512], in_=ob[:]
                    )

    nc.compile()
    return nc


def make_in_maps(x, W_attn, b_attn, W_proj, b_proj):
    bf16 = ml_dtypes.bfloat16
    in_maps = []
    for i in range(NCORES):
        b, g = i // 2, i % 2
        j0 = g * JL
        in_maps.append({
            "xT": np.ascontiguousarray(x[b].T).astype(bf16),
            "wq": W_attn[:, j0:j0 + JL].astype(bf16),
            "wk": W_attn[:, E + j0:E + j0 + JL].astype(bf16),
            "wv": W_attn[:, 2 * E + j0:2 * E + j0 + JL].astype(bf16),
            "wp": W_proj[j0:j0 + JL, :].astype(bf16),
            "bq": np.ascontiguousarray(
                b_attn[j0:j0 + JL].astype(np.float32).reshape(JT, P).T),
            "bk": np.ascontiguousarray(
                b_attn[E + j0:E + j0 + JL].astype(np.float32).reshape(JT, P).T),
        })
    return in_maps


def kernel(x, W_attn, b_attn, W_proj, b_proj):
    global _NC_CACHE
    if _NC_CACHE is None:
        _NC_CACHE = build_nc()
    nc = _NC_CACHE

    in_maps = make_in_maps(x, W_attn, b_attn, W_proj, b_proj)
    res = run_bass_kernel_spmd(nc, in_maps, core_ids=list(range(NCORES)))

    # host unshard: sum the two head-group partials + exact bias corrections
    bias_row = b_proj.astype(np.float32).copy()
    for g in range(2):
        j0 = g * JL
        bv = b_attn[2 * E + j0:2 * E + j0 + JL].astype(np.float32)
        bias_row += bv @ W_proj[j0:j0 + JL, :].astype(np.float32)

    full = np.empty((B, S, E), np.float32)
    for b in range(B):
        full[b] = (res.results[2 * b]["out"] + res.results[2 * b + 1]["out"]
                   + bias_row[None, :])
    return full


# revision 23
# speedup vs baseline: 1.3223x; 1.0071x over previous
"""Causal multi-head attention block (B=4, S=1024, E=1024, H=16, D=64) on 8 TRN2 cores.

Sharding: data-parallel over batch (4) x tensor-parallel over heads (2 groups of 8).
Core i handles batch i//2, head-group i%2. Each core computes its partial output
projection (row-parallel W_proj); the host sums the two TP partials per batch and
applies the (exact) bias corrections.

Device-side math per core (bf16 compute, f32 accumulate):
  qT = (Wq_g)^T x^T + bq_g          [512, 1024]  (head-major rows h*64+d)
  kT = (Wk_g)^T x^T + bk_g          [512, 1024]
  v  = x Wv_g                       [1024, 512]  (+ ones column per head -> denominator)
  For each head h: PT[sk, sq] = exp((kT_h^T qT_h)/8) * causal_mask (lower blocks only)
  o2T_h[d, sq] = sum_sk v_h[sk, d] * PT[sk, sq];  denom[sq] = ones-row
  o2T_h /= denom  (softmax normalize; no max subtraction -- logits are O(1))
  out_partial = o2T^T Wp_g          [1024, 1024]
Host: out[b] = out_partial[2b] + out_partial[2b+1] + (bv_0 Wp_0 + bv_1 Wp_1 + b_proj)
(the v-bias term is exact because softmax rows sum to 1).
"""

import numpy as np
import ml_dtypes

import concourse.bass as bass
import concourse.tile as tile
from concourse import bacc, mybir
from concourse.bass_utils import run_bass_kernel_spmd
from concourse.masks import make_upper_triangular

BF16 = mybir.dt.bfloat16
F32 = mybir.dt.float32

B, S, E = 4, 1024, 1024
H_TOT, D = 16, 64
NCORES = 8
HL = 8            # heads per core
JL = HL * D       # 512 local qkv dim
P = 128
ET = E // P       # 8 k-tiles over embed dim
JT = JL // P      # 4 partition-tiles over local qkv dim

_NC_CACHE = None


def build_nc():
    nc = bacc.Bacc()

    xT = nc.declare_dram_parameter("xT", [E, S], BF16, isOutput=False)
    wq = nc.declare_dram_parameter("wq", [E, JL], BF16, isOutput=False)
    wk = nc.declare_dram_parameter("wk", [E, JL], BF16, isOutput=False)
    wv = nc.declare_dram_parameter("wv", [E, JL], BF16, isOutput=False)
    wp = nc.declare_dram_parameter("wp", [JL, E], BF16, isOutput=False)
    bq = nc.declare_dram_parameter("bq", [P, JT], F32, isOutput=False)
    bk = nc.declare_dram_parameter("bk", [P, JT], F32, isOutput=False)
    out = nc.declare_dram_parameter("out", [S, E], F32, isOutput=True)

    with tile.TileContext(nc) as tc:
        with (
            tc.tile_pool(name="singles", bufs=1) as singles,
            tc.tile_pool(name="pt", bufs=4) as pt_pool,
            tc.tile_pool(name="rec", bufs=2) as rec_pool,
            tc.tile_pool(name="bc", bufs=2) as bc_pool,
            tc.tile_pool(name="outst", bufs=3) as out_pool,
            tc.tile_pool(name="ps_mm", bufs=2, space="PSUM") as ps_mm,
            tc.tile_pool(name="ps_l", bufs=2, space="PSUM") as ps_l,
            tc.tile_pool(name="ps_o", bufs=2, space="PSUM") as ps_o,
        ):
            # ---- static inputs -> SBUF (per-ktile DMAs so the first QKV
            # chain can start as soon as its first tiles land) ----
            xT_sb = singles.tile([P, ET, S], BF16)
            wq_sb = singles.tile([P, ET, JL], BF16)
            wk_sb = singles.tile([P, ET, JL], BF16)
            wv_sb = singles.tile([P, ET, JL], BF16)
            xT_r = xT[:, :].rearrange("(o p) s -> p o s", p=P)
            wq_r = wq[:, :].rearrange("(o p) j -> p o j", p=P)
            wk_r = wk[:, :].rearrange("(o p) j -> p o j", p=P)
            wv_r = wv[:, :].rearrange("(o p) j -> p o j", p=P)
            bq_sb = singles.tile([P, JT], F32)
            nc.sync.dma_start(out=bq_sb[:], in_=bq[:, :])
            bk_sb = singles.tile([P, JT], F32)
            nc.sync.dma_start(out=bk_sb[:], in_=bk[:, :])
            for kt in range(ET):
                nc.sync.dma_start(out=wq_sb[:, kt], in_=wq_r[:, kt])
                nc.sync.dma_start(out=xT_sb[:, kt], in_=xT_r[:, kt])
            for kt in range(ET):
                nc.sync.dma_start(out=wk_sb[:, kt], in_=wk_r[:, kt])
            for kt in range(ET):
                nc.sync.dma_start(out=wv_sb[:, kt], in_=wv_r[:, kt])
            wp_sb = singles.tile([P, JT, E], BF16)
            nc.sync.dma_start(out=wp_sb[:], in_=wp[:, :].rearrange("(o p) e -> p o e", p=P))

            # causal keep-mask for diagonal PT blocks: 1 where sq >= sk else 0
            mask_sb = singles.tile([P, P], BF16)
            make_upper_triangular(nc, mask_sb[:], val=1.0, diag=True)

            # ---- QKV projections ----
            qT_sb = singles.tile([P, JT, S], BF16)   # row j = h*64+d, head-major
            kT_sb = singles.tile([P, JT, S], BF16)
            o2T_sb = singles.tile([P, JT, S], BF16)  # normalized attn out, same row layout
            vaug_sb = singles.tile([P, ET, HL, D + 1], BF16)  # [sk_p, sk_tile, head, d|ones]
            nc.vector.memset(vaug_sb[:, :, :, D:D + 1], 1.0)

            for jt in range(JT):
                for w_sb, b_sb, dst in ((wq_sb, bq_sb, qT_sb), (wk_sb, bk_sb, kT_sb)):
                    for nb in range(2):
                        ps = ps_mm.tile([P, 512], F32, tag="mm")
                        for kt in range(ET):
                            nc.tensor.matmul(
                                ps[:],
                                lhsT=w_sb[:, kt, jt * P:(jt + 1) * P],
                                rhs=xT_sb[:, kt, nb * 512:(nb + 1) * 512],
                                start=(kt == 0), stop=(kt == ET - 1),
                            )
                        # ScalarE is idle during the QKV phase; do the
                        # psum->sbuf bias-add+cast there, not on DVE
                        nc.scalar.activation(
                            out=dst[:, jt, nb * 512:(nb + 1) * 512], in_=ps[:],
                            func=mybir.ActivationFunctionType.Identity,
                            bias=b_sb[:, jt:jt + 1],
                        )
            for st in range(ET):
                ps = ps_mm.tile([P, 512], F32, tag="mm")
                for kt in range(ET):
                    nc.tensor.matmul(
                        ps[:],
                        lhsT=xT_sb[:, kt, st * P:(st + 1) * P],
                        rhs=wv_sb[:, kt, :],
                        start=(kt == 0), stop=(kt == ET - 1),
                    )
                nc.scalar.copy(
                    out=vaug_sb[:, st, :, 0:D],
                    in_=ps[:].rearrange("p (h d) -> p h d", h=HL),
                )

            # ---- attention, processed in head PAIRS: head 2p sits in array
            # rows 0-63 and head 2p+1 in rows 64-127, so their K=64 QK^T
            # matmuls run CONCURRENTLY in the two row-halves (keeps the
            # array fully active -> HAM stays at the 2.4 GHz clock).
            # Pair p+1's QK overlaps pair p's PV/normalize (one-pair
            # lookahead keeps PE fed while ScalarE exps). ----
            def emit_qk_pair(h0):
                views = []
                for hh in (h0, h0 + 1):
                    jt0, po = hh // 2, (hh % 2) * 64
                    views.append((
                        qT_sb[po:po + 64, jt0, :],
                        kT_sb[po:po + 64, jt0, :],
                        pt_pool.tile([P, ET, S], BF16, tag="pt", name=f"pt_{hh}"),
                    ))
                for t in range(ET):
                    lo = t * P
                    psls = [ps_l.tile([P, 1024], F32, tag="psl", name=f"psl_{t}_{j}") for j in range(2)]
                    for cb in range(2):
                        c0, c1 = cb * 512, (cb + 1) * 512
                        s0 = max(lo, c0)
                        if s0 >= c1:
                            continue
                        # back-to-back row-half matmuls execute concurrently
                        for (qh, kh, _), psl in zip(views, psls):
                            nc.tensor.matmul(
                                psl[:, s0:c1],
                                lhsT=kh[:, lo:lo + P],
                                rhs=qh[:, s0:c1],
                                start=True, stop=True,
                            )
                    for (_, _, pT), psl in zip(views, psls):
                        nc.scalar.activation(
                            out=pT[:, t, lo:S], in_=psl[:, lo:S],
                            func=mybir.ActivationFunctionType.Exp, scale=0.125,
                        )
                for _, _, pT in views:
                    # mask all 8 diagonal blocks in one strided multiply:
                    # block t is at free offset t*(S+P) in the flattened tile
                    diag = bass.AP(tensor=pT.tensor, offset=pT.offset,
                                   ap=[list(pT.ap[0]), [S + P, ET], [1, P]])
                    nc.vector.tensor_mul(
                        out=diag, in0=diag,
                        in1=mask_sb[:, None, :].to_broadcast([P, ET, P]),
                    )
                return [v[2] for v in views]

            def emit_pv(h, pT):
                jt0, po = h // 2, (h % 2) * 64
                for sqb in range(2):
                    c0, c1 = sqb * 512, (sqb + 1) * 512
                    pso = ps_o.tile([P, 512], F32, tag="pso")
                    ts = [t for t in range(ET) if t * P < c1]
                    for i, t in enumerate(ts):
                        s0 = max(t * P, c0)
                        off = s0 - c0
                        nc.tensor.matmul(
                            pso[:D + 1, off:512],
                            lhsT=vaug_sb[:, t, h, :],
                            rhs=pT[:, t, s0:c1],
                            start=(i == 0), stop=(i == len(ts) - 1),
                            skip_group_check=True,
                        )
                    # normalize: o2T_h[:, c0:c1] = pso[:64] / pso[64]
                    rec = rec_pool.tile([P, 512], F32)
                    # custom-DVE op wants SBUF input at partition 0: stage the
                    # denominator row first, then approximate in place
                    nc.vector.tensor_copy(out=rec[:1, :], in_=pso[D:D + 1, :])
                    nc.vector.reciprocal_approx_fast(out=rec[:1, :], in_=rec[:1, :])
                    bcst = bc_pool.tile([P, 512], F32)
                    nc.gpsimd.partition_broadcast(bcst[:64, :], rec[:1, :])
                    nc.vector.tensor_mul(
                        out=o2T_sb[po:po + 64, jt0, c0:c1],
                        in0=pso[:64, :], in1=bcst[:64, :],
                    )

            pair_prev = emit_qk_pair(0)
            for p in range(1, HL // 2):
                pair_cur = emit_qk_pair(2 * p)
                emit_pv(2 * p - 2, pair_prev[0])
                emit_pv(2 * p - 1, pair_prev[1])
                pair_prev = pair_cur
            emit_pv(HL - 2, pair_prev[0])
            emit_pv(HL - 1, pair_prev[1])

            # ---- output projection (partial over local heads) ----
            for st in range(S // P):
                for eb in range(2):
                    psf = ps_mm.tile([P, 512], F32, tag="mm")
                    for kt in range(JT):
                        nc.tensor.matmul(
                            psf[:],
                            lhsT=o2T_sb[:, kt, st * P:(st + 1) * P],
                            rhs=wp_sb[:, kt, eb * 512:(eb + 1) * 512],
                            start=(kt == 0), stop=(kt == JT - 1),
                        )
                    ob = out_pool.tile([P, 512], F32)
                    nc.scalar.copy(out=ob[:], in_=psf[:])
                    nc.sync.dma_start(
                        out=out[st * P:(st + 1) * P, eb * 512:(eb + 1) * 512], in_=ob[:]
                    )

    nc.compile()
    return nc


def make_in_maps(x, W_attn, b_attn, W_proj, b_proj):
    bf16 = ml_dtypes.bfloat16
    in_maps = []
    for i in range(NCORES):
        b, g = i // 2, i % 2
        j0 = g * JL
        in_maps.append({
            "xT": np.ascontiguousarray(x[b].T).astype(bf16),
            "wq": W_attn[:, j0:j0 + JL].astype(bf16),
            "wk": W_attn[:, E + j0:E + j0 + JL].astype(bf16),
            "wv": W_attn[:, 2 * E + j0:2 * E + j0 + JL].astype(bf16),
            "wp": W_proj[j0:j0 + JL, :].astype(bf16),
            "bq": np.ascontiguousarray(
                b_attn[j0:j0 + JL].astype(np.float32).reshape(JT, P).T),
            "bk": np.ascontiguousarray(
                b_attn[E + j0:E + j0 + JL].astype(np.float32).reshape(JT, P).T),
        })
    return in_maps


def kernel(x, W_attn, b_attn, W_proj, b_proj):
    global _NC_CACHE
    if _NC_CACHE is None:
        _NC_CACHE = build_nc()
    nc = _NC_CACHE

    in_maps = make_in_maps(x, W_attn, b_attn, W_proj, b_proj)
    res = run_bass_kernel_spmd(nc, in_maps, core_ids=list(range(NCORES)))

    # host unshard: sum the two head-group partials + exact bias corrections
    bias_row = b_proj.astype(np.float32).copy()
    for g in range(2):
        j0 = g * JL
        bv = b_attn[2 * E + j0:2 * E + j0 + JL].astype(np.float32)
        bias_row += bv @ W_proj[j0:j0 + JL, :].astype(np.float32)

    full = np.empty((B, S, E), np.float32)
    for b in range(B):
        full[b] = (res.results[2 * b]["out"] + res.results[2 * b + 1]["out"]
                   + bias_row[None, :])
    return full


# revision 26
# speedup vs baseline: 1.4112x; 1.0672x over previous
"""Causal multi-head attention block (B=4, S=1024, E=1024, H=16, D=64) on 8 TRN2 cores.

Sharding: data-parallel over batch (4) x tensor-parallel over heads (2 groups of 8).
Core i handles batch i//2, head-group i%2. Each core computes its partial output
projection (row-parallel W_proj); the host sums the two TP partials per batch and
applies the (exact) bias corrections.

Device-side math per core (bf16 compute, f32 accumulate):
  qT = (Wq_g)^T x^T + bq_g          [512, 1024]  (head-major rows h*64+d)
  kT = (Wk_g)^T x^T + bk_g          [512, 1024]
  v  = x Wv_g                       [1024, 512]  (+ ones column per head -> denominator)
  For each head h: PT[sk, sq] = exp((kT_h^T qT_h)/8) * causal_mask (lower blocks only)
  o2T_h[d, sq] = sum_sk v_h[sk, d] * PT[sk, sq];  denom[sq] = ones-row
  o2T_h /= denom  (softmax normalize; no max subtraction -- logits are O(1))
  out_partial = o2T^T Wp_g          [1024, 1024]
Host: out[b] = out_partial[2b] + out_partial[2b+1] + (bv_0 Wp_0 + bv_1 Wp_1 + b_proj)
(the v-bias term is exact because softmax rows sum to 1).
"""

import numpy as np
import ml_dtypes

import concourse.bass as bass
import concourse.tile as tile
from concourse import bacc, mybir
from concourse.bass_utils import run_bass_kernel_spmd
from concourse.masks import make_upper_triangular

BF16 = mybir.dt.bfloat16
F32 = mybir.dt.float32

B, S, E = 4, 1024, 1024
H_TOT, D = 16, 64
NCORES = 8
HL = 8            # heads per core
JL = HL * D       # 512 local qkv dim
P = 128
ET = E // P       # 8 k-tiles over embed dim
JT = JL // P      # 4 partition-tiles over local qkv dim

_NC_CACHE = None


def build_nc():
    nc = bacc.Bacc()

    xT = nc.declare_dram_parameter("xT", [E, S], BF16, isOutput=False)
    wq = nc.declare_dram_parameter("wq", [E, JL], BF16, isOutput=False)
    wk = nc.declare_dram_parameter("wk", [E, JL], BF16, isOutput=False)
    wv = nc.declare_dram_parameter("wv", [E, JL], BF16, isOutput=False)
    wp = nc.declare_dram_parameter("wp", [JL, E], BF16, isOutput=False)
    bq = nc.declare_dram_parameter("bq", [P, JT], F32, isOutput=False)
    bk = nc.declare_dram_parameter("bk", [P, JT], F32, isOutput=False)
    out = nc.declare_dram_parameter("out", [S, E], F32, isOutput=True)

    with tile.TileContext(nc) as tc:
        with (
            tc.tile_pool(name="singles", bufs=1) as singles,
            tc.tile_pool(name="pt", bufs=4) as pt_pool,
            tc.tile_pool(name="rec", bufs=2) as rec_pool,
            tc.tile_pool(name="bc", bufs=2) as bc_pool,
            tc.tile_pool(name="outst", bufs=3) as out_pool,
            tc.tile_pool(name="ps_mm", bufs=2, space="PSUM") as ps_mm,
            tc.tile_pool(name="ps_l", bufs=2, space="PSUM") as ps_l,
            tc.tile_pool(name="ps_o", bufs=2, space="PSUM") as ps_o,
        ):
            # ---- static inputs -> SBUF (per-ktile DMAs so the first QKV
            # chain can start as soon as its first tiles land) ----
            xT_sb = singles.tile([P, ET, S], BF16)
            wq_sb = singles.tile([P, ET, JL], BF16)
            wk_sb = singles.tile([P, ET, JL], BF16)
            wv_sb = singles.tile([P, ET, JL], BF16)
            xT_r = xT[:, :].rearrange("(o p) s -> p o s", p=P)
            wq_r = wq[:, :].rearrange("(o p) j -> p o j", p=P)
            wk_r = wk[:, :].rearrange("(o p) j -> p o j", p=P)
            wv_r = wv[:, :].rearrange("(o p) j -> p o j", p=P)
            bq_sb = singles.tile([P, JT], F32)
            nc.sync.dma_start(out=bq_sb[:], in_=bq[:, :])
            bk_sb = singles.tile([P, JT], F32)
            nc.sync.dma_start(out=bk_sb[:], in_=bk[:, :])
            for kt in range(ET):
                nc.sync.dma_start(out=wq_sb[:, kt], in_=wq_r[:, kt])
                nc.sync.dma_start(out=wk_sb[:, kt], in_=wk_r[:, kt])
                nc.sync.dma_start(out=xT_sb[:, kt], in_=xT_r[:, kt])
            for kt in range(ET):
                nc.sync.dma_start(out=wv_sb[:, kt], in_=wv_r[:, kt])
            wp_sb = singles.tile([P, JT, E], BF16)
            nc.sync.dma_start(out=wp_sb[:], in_=wp[:, :].rearrange("(o p) e -> p o e", p=P))

            # causal keep-mask for diagonal PT blocks: 1 where sq >= sk else 0
            mask_sb = singles.tile([P, P], BF16)
            make_upper_triangular(nc, mask_sb[:], val=1.0, diag=True)

            # ---- QKV projections ----
            qT_sb = singles.tile([P, JT, S], BF16)   # row j = h*64+d, head-major
            kT_sb = singles.tile([P, JT, S], BF16)
            o2T_sb = singles.tile([P, JT, S], BF16)  # normalized attn out, same row layout
            vaug_sb = singles.tile([P, ET, HL, D + 1], BF16)  # [sk_p, sk_tile, head, d|ones]
            nc.vector.memset(vaug_sb[:, :, :, D:D + 1], 1.0)

            def emit_qk_chains(jt):
                for w_sb, b_sb, dst in ((wq_sb, bq_sb, qT_sb), (wk_sb, bk_sb, kT_sb)):
                    for nb in range(2):
                        ps = ps_mm.tile([P, 512], F32, tag="mm", name=f"mm_{jt}_{nb}")
                        for kt in range(ET):
                            nc.tensor.matmul(
                                ps[:],
                                lhsT=w_sb[:, kt, jt * P:(jt + 1) * P],
                                rhs=xT_sb[:, kt, nb * 512:(nb + 1) * 512],
                                start=(kt == 0), stop=(kt == ET - 1),
                            )
                        nc.vector.tensor_scalar_add(
                            dst[:, jt, nb * 512:(nb + 1) * 512], ps[:],
                            b_sb[:, jt:jt + 1],
                        )

            def emit_v_chains():
                for st in range(ET):
                    ps = ps_mm.tile([P, 512], F32, tag="mm", name=f"mmv_{st}")
                    for kt in range(ET):
                        nc.tensor.matmul(
                            ps[:],
                            lhsT=xT_sb[:, kt, st * P:(st + 1) * P],
                            rhs=wv_sb[:, kt, :],
                            start=(kt == 0), stop=(kt == ET - 1),
                        )
                    nc.scalar.copy(
                        out=vaug_sb[:, st, :, 0:D],
                        in_=ps[:].rearrange("p (h d) -> p h d", h=HL),
                    )

            # ---- attention, processed in head PAIRS: head 2p sits in array
            # rows 0-63 and head 2p+1 in rows 64-127, so their K=64 QK^T
            # matmuls run CONCURRENTLY in the two row-halves (keeps the
            # array fully active -> HAM stays at the 2.4 GHz clock).
            # Pair p+1's QK overlaps pair p's PV/normalize (one-pair
            # lookahead keeps PE fed while ScalarE exps). ----
            def emit_qk_pair(h0):
                views = []
                for hh in (h0, h0 + 1):
                    jt0, po = hh // 2, (hh % 2) * 64
                    views.append((
                        qT_sb[po:po + 64, jt0, :],
                        kT_sb[po:po + 64, jt0, :],
                        pt_pool.tile([P, ET, S], BF16, tag="pt", name=f"pt_{hh}"),
                    ))
                for t in range(ET):
                    lo = t * P
                    psls = [ps_l.tile([P, 1024], F32, tag="psl", name=f"psl_{t}_{j}") for j in range(2)]
                    for cb in range(2):
                        c0, c1 = cb * 512, (cb + 1) * 512
                        s0 = max(lo, c0)
                        if s0 >= c1:
                            continue
                        # back-to-back row-half matmuls execute concurrently
                        for (qh, kh, _), psl in zip(views, psls):
                            nc.tensor.matmul(
                                psl[:, s0:c1],
                                lhsT=kh[:, lo:lo + P],
                                rhs=qh[:, s0:c1],
                                start=True, stop=True,
                            )
                    for (_, _, pT), psl in zip(views, psls):
                        nc.scalar.activation(
                            out=pT[:, t, lo:S], in_=psl[:, lo:S],
                            func=mybir.ActivationFunctionType.Exp, scale=0.125,
                        )
                for _, _, pT in views:
                    # mask all 8 diagonal blocks in one strided multiply:
                    # block t is at free offset t*(S+P) in the flattened tile
                    diag = bass.AP(tensor=pT.tensor, offset=pT.offset,
                                   ap=[list(pT.ap[0]), [S + P, ET], [1, P]])
                    nc.vector.tensor_mul(
                        out=diag, in0=diag,
                        in1=mask_sb[:, None, :].to_broadcast([P, ET, P]),
                    )
                return [v[2] for v in views]

            def emit_pv(h, pT):
                jt0, po = h // 2, (h % 2) * 64
                for sqb in range(2):
                    c0, c1 = sqb * 512, (sqb + 1) * 512
                    pso = ps_o.tile([P, 512], F32, tag="pso")
                    ts = [t for t in range(ET) if t * P < c1]
                    for i, t in enumerate(ts):
                        s0 = max(t * P, c0)
                        off = s0 - c0
                        nc.tensor.matmul(
                            pso[:D + 1, off:512],
                            lhsT=vaug_sb[:, t, h, :],
                            rhs=pT[:, t, s0:c1],
                            start=(i == 0), stop=(i == len(ts) - 1),
                            skip_group_check=True,
                        )
                    # normalize: o2T_h[:, c0:c1] = pso[:64] / pso[64]
                    rec = rec_pool.tile([P, 512], F32)
                    # custom-DVE op wants SBUF input at partition 0: stage the
                    # denominator row first, then approximate in place
                    nc.vector.tensor_copy(out=rec[:1, :], in_=pso[D:D + 1, :])
                    nc.vector.reciprocal_approx_fast(out=rec[:1, :], in_=rec[:1, :])
                    bcst = bc_pool.tile([P, 512], F32)
                    nc.gpsimd.partition_broadcast(bcst[:64, :], rec[:1, :])
                    nc.vector.tensor_mul(
                        out=o2T_sb[po:po + 64, jt0, c0:c1],
                        in0=pso[:64, :], in1=bcst[:64, :],
                    )

            # master pipeline: per segment p, emit the jt=p q/k projection
            # chains (full-array matmuls keep the HAM clock warm), the head
            # pair p QK^T, the v chains (once, at p=1), and the PREVIOUS
            # pair's PV + normalize. The scheduler interleaves by readiness;
            # the dense supply of full 128x128 matmuls keeps the PE at
            # 2.4 GHz while the half-array QK/PV ride along.
            emit_qk_chains(0)
            pair_prev = emit_qk_pair(0)
            for p in range(1, JT):
                emit_qk_chains(p)
                pair_cur = emit_qk_pair(2 * p)
                if p == 1:
                    emit_v_chains()
                emit_pv(2 * p - 2, pair_prev[0])
                emit_pv(2 * p - 1, pair_prev[1])
                pair_prev = pair_cur
            emit_pv(HL - 2, pair_prev[0])
            emit_pv(HL - 1, pair_prev[1])

            # ---- output projection (partial over local heads) ----
            for st in range(S // P):
                for eb in range(2):
                    psf = ps_mm.tile([P, 512], F32, tag="mm")
                    for kt in range(JT):
                        nc.tensor.matmul(
                            psf[:],
                            lhsT=o2T_sb[:, kt, st * P:(st + 1) * P],
                            rhs=wp_sb[:, kt, eb * 512:(eb + 1) * 512],
                            start=(kt == 0), stop=(kt == JT - 1),
                        )
                    ob = out_pool.tile([P, 512], F32)
                    nc.scalar.copy(out=ob[:], in_=psf[:])
                    nc.sync.dma_start(
                        out=out[st * P:(st + 1) * P, eb * 512:(eb + 1) * 512], in_=ob[:]
                    )

    nc.compile()
    return nc


def make_in_maps(x, W_attn, b_attn, W_proj, b_proj):
    bf16 = ml_dtypes.bfloat16
    in_maps = []
    for i in range(NCORES):
        b, g = i // 2, i % 2
        j0 = g * JL
        in_maps.append({
            "xT": np.ascontiguousarray(x[b].T).astype(bf16),
            "wq": W_attn[:, j0:j0 + JL].astype(bf16),
            "wk": W_attn[:, E + j0:E + j0 + JL].astype(bf16),
            "wv": W_attn[:, 2 * E + j0:2 * E + j0 + JL].astype(bf16),
            "wp": W_proj[j0:j0 + JL, :].astype(bf16),
            "bq": np.ascontiguousarray(
                b_attn[j0:j0 + JL].astype(np.float32).reshape(JT, P).T),
            "bk": np.ascontiguousarray(
                b_attn[E + j0:E + j0 + JL].astype(np.float32).reshape(JT, P).T),
        })
    return in_maps


def kernel(x, W_attn, b_attn, W_proj, b_proj):
    global _NC_CACHE
    if _NC_CACHE is None:
        _NC_CACHE = build_nc()
    nc = _NC_CACHE

    in_maps = make_in_maps(x, W_attn, b_attn, W_proj, b_proj)
    res = run_bass_kernel_spmd(nc, in_maps, core_ids=list(range(NCORES)))

    # host unshard: sum the two head-group partials + exact bias corrections
    bias_row = b_proj.astype(np.float32).copy()
    for g in range(2):
        j0 = g * JL
        bv = b_attn[2 * E + j0:2 * E + j0 + JL].astype(np.float32)
        bias_row += bv @ W_proj[j0:j0 + JL, :].astype(np.float32)

    full = np.empty((B, S, E), np.float32)
    for b in range(B):
        full[b] = (res.results[2 * b]["out"] + res.results[2 * b + 1]["out"]
                   + bias_row[None, :])
    return full
